# revision 2
# baseline (speedup 1.0000x reference)
# Braak-aware attention kernel for Trainium2 (Bass/Tile), 8 NeuronCores.
#
# Problem (per sample b of B=8, all fp32 in HBM):
#   bias[s]   = braak_embed[braak_stages[b], s]          (per-row constant)
#   q'[s,d]   = query[b,s,d] + bias[s]
#   S[s,t]    = sum_d q'[s,d] * key[b,t,d]
#   P         = softmax_t(S)
#   out[s,d]  = sum_t P[s,t] * value[b,t,d]
#
# Sharding: data-parallel, one sample per core (8 samples, 8 cores), no comms.
# The braak_embed gather by integer stage is host-side (pure indexing).
#
# Device strategy (v16; v15 measured 84.4us, see git-less history in comments):
#   Trace model (validated on v15): exec ~= [HAM flip at ~16.6us] +
#   post-flip PE columns / 2.34 cols-per-ns + tail + ~9us fixed postamble
#   (compiler sem-reset sweep). The PE array runs at ~0.9 col/ns before the
#   HAM clock flip (~9.3us after first matmul activity) and 2.34 after, and
#   is the bottleneck end-to-end, so v16 minimizes TOTAL PE columns and
#   keeps the array dense with real work:
#   - P^T via the DMA xbar transpose (one dma_start(transpose=True) per
#     s-tile, out AP [p, j, s]) instead of 64 PE transpose matmuls: -8192
#     PE columns and frees 2 PSUM banks.
#   - Wavefront accumulates 3.5 s-tiles (s0-s2 full + s3 half) in all 8
#     PSUM banks as Q^T/K^T d-tiles stream in. The leftover matmul backlog
#     at load-end plus the s3-second-half bridge REPLACES the old filler
#     matmuls: the array never idles at the wavefront->softmax handoff, so
#     the HAM gate stays up without burning ~12k filler columns.
#   - bias ships as [1, S] fp16 (2KB) and is partition-broadcast on GPSIMD,
#     taking 254KB off the DMA-capped load phase (both hw queues run at the
#     ~358 GB/s/core HBM cap during the load; Q/K land ~0.7us earlier).
#   - softmax order s0, s3, s1, s2 frees the two half banks for the first
#     AV quickly; AVs run in the same order. AV accumulates fp32 in
#     [128,512] half-bank tiles, double-buffered so the ACT normalize
#     (COPY x 1/rowsum) of half h overlaps the next half's matmuls.
#   - steady state alternates scores(s4..s7) and AVs on the PE; softmax
#     (DVE max / ACT exp) and the P^T DMA ride under them.
# Numerics: fp16 rounding of Q'/K dominates (~2.4e-3 output rel-L2 vs the
# fp32 reference; threshold 2e-2). The DMA transpose is exact byte movement.

import os
import sys

for _p in ("/opt/trn_rl_repo",):
    if _p not in sys.path:
        sys.path.insert(0, _p)

import numpy as np

import concourse.bass as bass
import concourse.tile as tile
from concourse import bacc, mybir
from concourse.bass_utils import run_bass_kernel_spmd

B, S, D = 8, 1024, 1024
P = 128
NT = S // P  # 8 tiles of 128 along every axis
F32 = mybir.dt.float32
F16 = mybir.dt.float16
EXP = mybir.ActivationFunctionType.Exp
COPY = mybir.ActivationFunctionType.Copy

N_WARM = 16  # identity warmup matmuls: start PE activity (HAM ramp) early


_CACHE = {}


def _build(ctx, tc):
    from concourse.alu_op_type import AluOpType

    nc = tc.nc
    qT_d = nc.dram_tensor("qT", [D, S], F16, kind="ExternalInput").ap()
    kT_d = nc.dram_tensor("kT", [D, S], F16, kind="ExternalInput").ap()
    v_d = nc.dram_tensor("v", [S, D], F16, kind="ExternalInput").ap()
    bias_d = nc.dram_tensor("bias1", [1, S], F16, kind="ExternalInput").ap()
    out_d = nc.dram_tensor("out", [S, D], F16, kind="ExternalOutput").ap()

    const = ctx.enter_context(tc.tile_pool(name="const", bufs=1))
    wts = ctx.enter_context(tc.tile_pool(name="wts", bufs=1))
    ppool = ctx.enter_context(tc.tile_pool(name="ppool", bufs=3))
    ptpool = ctx.enter_context(tc.tile_pool(name="ptpool", bufs=4))
    outpool = ctx.enter_context(tc.tile_pool(name="outpool", bufs=4))
    smalls = ctx.enter_context(tc.tile_pool(name="smalls", bufs=3))
    # all 8 PSUM banks: 3 x [128,1024] full score tiles + 2 x [128,512]
    # half-bank tiles (s3's two halves during the wavefront, AV halves after)
    psum_big = ctx.enter_context(tc.tile_pool(name="psum_big", bufs=3, space="PSUM"))
    psum_half = ctx.enter_context(tc.tile_pool(name="psum_half", bufs=2, space="PSUM"))

    # ---- constants; memset-fed warmup source lets PE warmup start in the
    # preamble without waiting on any DMA ----
    wsrc = const.tile([P, P], F16, tag="wsrc")
    nc.vector.memset(wsrc, 0.25)
    bias1 = const.tile([1, S], F16, tag="bias1")
    nc.sync.dma_start(out=bias1, in_=bias_d)
    bias_bc = const.tile([P, S], F16, tag="bias_bc")
    nc.gpsimd.partition_broadcast(bias_bc, bias1)

    # ---- PE warmup (no DMA deps): starts the HAM clock ramp ASAP. Writes
    # into the first psum_big slot; the wavefront's s2 reclaims it later.
    warm = psum_big.tile([P, S], F32, tag="sp", name="warm")
    for w in range(N_WARM):
        nc.tensor.matmul(
            warm[:, 0:P], wsrc, wsrc, start=(w == 0), stop=(w == N_WARM - 1)
        )

    # ---- persistent operands, one tile per 128-row d/t-tile: Tile deps are
    # tile-granular, and per-tile DMAs keep many transfers in flight (the
    # two hw queues together run at the per-core HBM cap ~358 GB/s). ----
    kt_t = [wts.tile([P, S], F16, tag=f"kt{c}", name=f"kt{c}") for c in range(NT)]
    qraw_t = [
        wts.tile([P, S], F16, tag=f"qraw{c}", name=f"qraw{c}") for c in range(NT)
    ]
    qb_t = [wts.tile([P, S], F16, tag=f"qb{c}", name=f"qb{c}") for c in range(NT)]
    vf_t = [wts.tile([P, D], F16, tag=f"vf{j}", name=f"vf{j}") for j in range(NT)]

    for c in range(NT):
        nc.scalar.dma_start(out=kt_t[c], in_=kT_d[c * P : (c + 1) * P, :])
        nc.sync.dma_start(out=qraw_t[c], in_=qT_d[c * P : (c + 1) * P, :])
        nc.vector.tensor_add(out=qb_t[c], in0=qraw_t[c], in1=bias_bc)
    # Throwaway ACTIVATE after the kt issues: hoists ACT_TABLE_LOAD well
    # before the first Exp without delaying the kt DMAs.
    actwarm = const.tile([1, 1], F16, tag="actwarm")
    nc.scalar.copy(out=actwarm, in_=bias_bc[0:1, 0:1])
    # V split across both hw queues BEHIND qk (FIFO keeps qk first)
    for j in range(NT):
        eng = nc.sync if j % 2 == 0 else nc.scalar
        eng.dma_start(out=vf_t[j], in_=v_d[j * P : (j + 1) * P, :])

    def q_lhsT(c, i):
        return qb_t[c][:, i * P : (i + 1) * P]

    def k_rhs_half(c, h):
        return kt_t[c][:, h * 512 : (h + 1) * 512]

    # ---- wavefront: s0,s1,s2 full + s3 first half accumulate per arriving
    # d-tile. At slow (pre-HAM-flip) clock the PE falls behind the DMA; the
    # backlog it drains after load-end bridges the softmax handoff. ----
    sp0 = psum_big.tile([P, S], F32, tag="sp", name="sp0")
    sp1 = psum_big.tile([P, S], F32, tag="sp", name="sp1")
    sp2 = psum_big.tile([P, S], F32, tag="sp", name="sp2")
    sps = (sp0, sp1, sp2)
    s3a = psum_half.tile([P, 512], F32, tag="oph", name="s3a")
    for c in range(NT):
        for i in (0, 1, 2):
            lhsT = q_lhsT(c, i)
            for h in range(2):
                nc.tensor.matmul(
                    sps[i][:, h * 512 : (h + 1) * 512],
                    lhsT,
                    k_rhs_half(c, h),
                    start=(c == 0),
                    stop=(c == NT - 1),
                )
        nc.tensor.matmul(
            s3a,
            q_lhsT(c, 3),
            k_rhs_half(c, 0),
            start=(c == 0),
            stop=(c == NT - 1),
        )
    # bridge: s3's second half, real work while softmax(s0) runs
    s3b = psum_half.tile([P, 512], F32, tag="oph", name="s3b")
    for c in range(NT):
        nc.tensor.matmul(
            s3b,
            q_lhsT(c, 3),
            k_rhs_half(c, 1),
            start=(c == 0),
            stop=(c == NT - 1),
        )

    def stage_softmax(i, sp):
        negmax = smalls.tile([P, 1], F32, tag="negmax", name=f"negmax{i}")
        nc.vector.reduce_max(
            out=negmax, in_=sp, axis=mybir.AxisListType.X, negate=True
        )
        pexp = ppool.tile([P, S], F16, tag="pexp", name=f"pexp{i}")
        sumexp = smalls.tile([P, 1], F32, tag="sumexp", name=f"sumexp{i}")
        nc.scalar.activation(
            out=pexp, in_=sp, func=EXP, bias=negmax, scale=1.0, accum_out=sumexp
        )
        return pexp, sumexp

    def stage_softmax_halves(ha, hb):
        m0 = smalls.tile([P, 1], F32, tag="negmax", name="m3a")
        nc.vector.reduce_max(out=m0, in_=ha, axis=mybir.AxisListType.X, negate=True)
        m1 = smalls.tile([P, 1], F32, tag="negmax", name="m3b")
        nc.vector.reduce_max(out=m1, in_=hb, axis=mybir.AxisListType.X, negate=True)
        negmax = smalls.tile([P, 1], F32, tag="negmax", name="m3")
        nc.vector.tensor_tensor(out=negmax, in0=m0, in1=m1, op=AluOpType.min)
        pexp = ppool.tile([P, S], F16, tag="pexp", name="pexp3")
        se0 = smalls.tile([P, 1], F32, tag="sumexp", name="se3a")
        nc.scalar.activation(
            out=pexp[:, 0:512], in_=ha, func=EXP, bias=negmax, scale=1.0,
            accum_out=se0,
        )
        se1 = smalls.tile([P, 1], F32, tag="sumexp", name="se3b")
        nc.scalar.activation(
            out=pexp[:, 512:1024], in_=hb, func=EXP, bias=negmax, scale=1.0,
            accum_out=se1,
        )
        sumexp = smalls.tile([P, 1], F32, tag="sumexp", name="sumexp3")
        nc.vector.tensor_add(out=sumexp, in0=se0, in1=se1)
        return pexp, sumexp

    def stage_ptT(i, pexp):
        """P^T via the DMA xbar: one transpose DMA per s-tile. Out AP
        [p, j, s] scatters each 128x128 block transposed in place."""
        pt = ptpool.tile([P, S], F16, tag="pt", name=f"pt{i}")
        nc.sync.dma_start(
            out=pt[:, :].rearrange("p (j s) -> p j s", j=NT),
            in_=pexp[:, :],
            transpose=True,
        )
        return pt

    def stage_scores(i):
        sp = psum_big.tile([P, S], F32, tag="sp", name=f"sp{i}")
        for c in range(NT):
            lhsT = q_lhsT(c, i)
            for h in range(2):
                nc.tensor.matmul(
                    sp[:, h * 512 : (h + 1) * 512],
                    lhsT,
                    k_rhs_half(c, h),
                    start=(c == 0),
                    stop=(c == NT - 1),
                )
        return sp

    def stage_av(i, pt, sumexp):
        # Each half is its own PSUM tile + SBUF tile: the half-h normalize
        # and store overlap the half-(h+1) matmuls with no false WAR deps.
        recip = smalls.tile([P, 1], F32, tag="recip", name=f"recip{i}")
        nc.vector.reciprocal(out=recip, in_=sumexp)
        for h in range(2):
            op = psum_half.tile([P, 512], F32, tag="oph", name=f"op{i}_{h}")
            ot = outpool.tile([P, 512], F16, tag="ot", name=f"ot{i}_{h}")
            for j in range(NT):
                nc.tensor.matmul(
                    op,
                    pt[:, j * P : (j + 1) * P],
                    vf_t[j][:, h * 512 : (h + 1) * 512],
                    start=(j == 0),
                    stop=(j == NT - 1),
                )
            # normalize on ACT (per-partition scale); DVE stays light
            nc.scalar.activation(out=ot, in_=op, func=COPY, scale=recip)
            nc.sync.dma_start(
                out=out_d[i * P : (i + 1) * P, h * 512 : (h + 1) * 512], in_=ot
            )

    # ---- schedule. softmax/AV order: 0, 3, 1, 2, 4..7 (s3 early frees the
    # half banks for AV0; matching AV order keeps pt lifetimes short).
    sm = {}
    pts = {}
    sm[0] = stage_softmax(0, sp0)
    pts[0] = stage_ptT(0, sm[0][0])
    sm[3] = stage_softmax_halves(s3a, s3b)
    pts[3] = stage_ptT(3, sm[3][0])
    sm[1] = stage_softmax(1, sp1)
    pts[1] = stage_ptT(1, sm[1][0])
    sm[2] = stage_softmax(2, sp2)
    pts[2] = stage_ptT(2, sm[2][0])

    av_order = [0, 3, 1, 2, 4, 5, 6, 7]
    for k, i in enumerate(range(4, NT)):
        sp = stage_scores(i)
        sm[i] = stage_softmax(i, sp)
        pts[i] = stage_ptT(i, sm[i][0])
        j = av_order[k]
        stage_av(j, pts.pop(j), sm.pop(j)[1])
    for j in av_order[NT - 4 :]:
        stage_av(j, pts.pop(j), sm.pop(j)[1])


def _get_program():
    key = "v16"
    if key not in _CACHE:
        nc = bacc.Bacc("TRN2", num_devices=B)
        from contextlib import ExitStack

        with tile.TileContext(nc) as tc:
            with ExitStack() as ctx:
                _build(ctx, tc)
        nc.compile()
        _CACHE[key] = nc
    return _CACHE[key]


def kernel(query, key, value, braak_embed, braak_stages):
    query = np.asarray(query, dtype=np.float32)
    key_in = np.asarray(key, dtype=np.float32)
    value = np.asarray(value, dtype=np.float32)
    braak_embed = np.asarray(braak_embed, dtype=np.float32)
    stages = np.asarray(braak_stages).astype(np.int64)

    bias16 = braak_embed[stages].astype(np.float16)  # [B, S] host gather
    # Host marshalling: fp16 casts (the kernel consumes fp16 either way)
    # and layout transposes of Q/K to the d-major layout the PE needs.
    qT16 = np.ascontiguousarray(query.astype(np.float16).transpose(0, 2, 1))
    kT16 = np.ascontiguousarray(key_in.astype(np.float16).transpose(0, 2, 1))
    v16 = np.ascontiguousarray(value.astype(np.float16))

    nc = _get_program()
    in_maps = [
        {
            "qT": qT16[b],
            "kT": kT16[b],
            "v": v16[b],
            "bias1": bias16[b : b + 1],
        }
        for b in range(B)
    ]
    trace = os.environ.get("BRAAK_TRACE", "0") == "1"
    if trace:
        try:  # tracing needs the NTFF hook; never let it break a run
            from antenv.axon_hooks import get_axon_ntff_profile_hook  # noqa: F401
        except ImportError:
            trace = False
    res = run_bass_kernel_spmd(nc, in_maps, list(range(B)), trace=trace)
    if trace:
        kernel.last_exec_time_ns = res.exec_time_ns
        kernel.last_profile = res
    out = np.stack([res.results[b]["out"] for b in range(B)]).astype(np.float32)
    return out


kernel.last_exec_time_ns = None
kernel.last_profile = None


# revision 6
# speedup vs baseline: 1.1140x; 1.1140x over previous
# Braak-aware attention kernel for Trainium2 (Bass/Tile), 8 NeuronCores.
#
# Problem (per sample b of B=8, all fp32 in HBM):
#   bias[s]   = braak_embed[braak_stages[b], s]          (per-row constant)
#   q'[s,d]   = query[b,s,d] + bias[s]
#   S[s,t]    = sum_d q'[s,d] * key[b,t,d]
#   P         = softmax_t(S)
#   out[s,d]  = sum_t P[s,t] * value[b,t,d]
#
# Sharding: data-parallel, one sample per core (8 samples, 8 cores), no comms.
# The braak_embed gather by integer stage is host-side (pure indexing).
#
# Device strategy (v16; v15 measured 84.4us, see git-less history in comments):
#   Trace model (validated on v15): exec ~= [HAM flip at ~16.6us] +
#   post-flip PE columns / 2.34 cols-per-ns + tail + ~9us fixed postamble
#   (compiler sem-reset sweep). The PE array runs at ~0.9 col/ns before the
#   HAM clock flip (~9.3us after first matmul activity) and 2.34 after, and
#   is the bottleneck end-to-end, so v16 minimizes TOTAL PE columns and
#   keeps the array dense with real work:
#   - P^T via the DMA xbar transpose (one dma_start(transpose=True) per
#     s-tile, out AP [p, j, s]) instead of 64 PE transpose matmuls: -8192
#     PE columns and frees 2 PSUM banks.
#   - Wavefront accumulates 3.5 s-tiles (s0-s2 full + s3 half) in all 8
#     PSUM banks as Q^T/K^T d-tiles stream in. The leftover matmul backlog
#     at load-end plus the s3-second-half bridge REPLACES the old filler
#     matmuls: the array never idles at the wavefront->softmax handoff, so
#     the HAM gate stays up without burning ~12k filler columns.
#   - bias ships as [1, S] fp16 (2KB) and is partition-broadcast on GPSIMD,
#     taking 254KB off the DMA-capped load phase (both hw queues run at the
#     ~358 GB/s/core HBM cap during the load; Q/K land ~0.7us earlier).
#   - softmax order s0, s3, s1, s2 frees the two half banks for the first
#     AV quickly; AVs run in the same order. AV accumulates fp32 in
#     [128,512] half-bank tiles, double-buffered so the ACT normalize
#     (COPY x 1/rowsum) of half h overlaps the next half's matmuls.
#   - steady state alternates scores(s4..s7) and AVs on the PE; softmax
#     (DVE max / ACT exp) and the P^T DMA ride under them.
# Numerics: fp16 rounding of Q'/K dominates (~2.4e-3 output rel-L2 vs the
# fp32 reference; threshold 2e-2). The DMA transpose is exact byte movement.

import os
import sys

for _p in ("/opt/trn_rl_repo",):
    if _p not in sys.path:
        sys.path.insert(0, _p)

import numpy as np

import concourse.bass as bass
import concourse.tile as tile
from concourse import bacc, mybir
from concourse.bass_utils import run_bass_kernel_spmd

B, S, D = 8, 1024, 1024
P = 128
NT = S // P  # 8 tiles of 128 along every axis
F32 = mybir.dt.float32
F16 = mybir.dt.float16
EXP = mybir.ActivationFunctionType.Exp
COPY = mybir.ActivationFunctionType.Copy

N_WARM = 16  # identity warmup matmuls: start PE activity (HAM ramp) early


_CACHE = {}


def _build(ctx, tc):
    from concourse.alu_op_type import AluOpType

    nc = tc.nc
    qT_d = nc.dram_tensor("qT", [D, S], F16, kind="ExternalInput").ap()
    kT_d = nc.dram_tensor("kT", [D, S], F16, kind="ExternalInput").ap()
    v_d = nc.dram_tensor("v", [S, D], F16, kind="ExternalInput").ap()
    # bias pre-broadcast to 128 partitions host-side (a GPSIMD
    # partition_broadcast was tried: its ucode lib-load + drain serialized
    # ~13us before the first qb add — far worse than the 254KB of DMA)
    bias_d = nc.dram_tensor("biasb", [P, S], F16, kind="ExternalInput").ap()
    out_d = nc.dram_tensor("out", [S, D], F16, kind="ExternalOutput").ap()

    const = ctx.enter_context(tc.tile_pool(name="const", bufs=1))
    wts = ctx.enter_context(tc.tile_pool(name="wts", bufs=1))
    ppool = ctx.enter_context(tc.tile_pool(name="ppool", bufs=3))
    ptpool = ctx.enter_context(tc.tile_pool(name="ptpool", bufs=4))
    outpool = ctx.enter_context(tc.tile_pool(name="outpool", bufs=4))
    smalls = ctx.enter_context(tc.tile_pool(name="smalls", bufs=3))
    # all 8 PSUM banks: 3 x [128,1024] full score tiles + 2 x [128,512]
    # half-bank tiles (s3's two halves during the wavefront, AV halves after)
    psum_big = ctx.enter_context(tc.tile_pool(name="psum_big", bufs=3, space="PSUM"))
    psum_half = ctx.enter_context(tc.tile_pool(name="psum_half", bufs=2, space="PSUM"))

    # ---- constants; memset-fed warmup source lets PE warmup start in the
    # preamble without waiting on any DMA ----
    wsrc = const.tile([P, P], F16, tag="wsrc")
    nc.vector.memset(wsrc, 0.25)
    bias_bc = const.tile([P, S], F16, tag="bias_bc")
    nc.sync.dma_start(out=bias_bc, in_=bias_d)

    # ---- PE warmup (no DMA deps): starts the HAM clock ramp ASAP. Writes
    # into the first psum_big slot; the wavefront's s2 reclaims it later.
    warm = psum_big.tile([P, S], F32, tag="sp", name="warm")
    for w in range(N_WARM):
        nc.tensor.matmul(
            warm[:, 0:P], wsrc, wsrc, start=(w == 0), stop=(w == N_WARM - 1)
        )

    # ---- persistent operands, one tile per 128-row d/t-tile: Tile deps are
    # tile-granular, and per-tile DMAs keep many transfers in flight (the
    # two hw queues together run at the per-core HBM cap ~358 GB/s). ----
    kt_t = [wts.tile([P, S], F16, tag=f"kt{c}", name=f"kt{c}") for c in range(NT)]
    qraw_t = [
        wts.tile([P, S], F16, tag=f"qraw{c}", name=f"qraw{c}") for c in range(NT)
    ]
    qb_t = [wts.tile([P, S], F16, tag=f"qb{c}", name=f"qb{c}") for c in range(NT)]
    vf_t = [wts.tile([P, D], F16, tag=f"vf{j}", name=f"vf{j}") for j in range(NT)]

    for c in range(NT):
        nc.scalar.dma_start(out=kt_t[c], in_=kT_d[c * P : (c + 1) * P, :])
        nc.sync.dma_start(out=qraw_t[c], in_=qT_d[c * P : (c + 1) * P, :])
        nc.vector.tensor_add(out=qb_t[c], in0=qraw_t[c], in1=bias_bc)
    # Throwaway ACTIVATE after the kt issues: hoists ACT_TABLE_LOAD well
    # before the first Exp without delaying the kt DMAs.
    actwarm = const.tile([1, 1], F16, tag="actwarm")
    nc.scalar.copy(out=actwarm, in_=bias_bc[0:1, 0:1])
    # V split across both hw queues BEHIND qk (FIFO keeps qk first)
    for j in range(NT):
        eng = nc.sync if j % 2 == 0 else nc.scalar
        eng.dma_start(out=vf_t[j], in_=v_d[j * P : (j + 1) * P, :])

    def q_lhsT(c, i):
        return qb_t[c][:, i * P : (i + 1) * P]

    def k_rhs_half(c, h):
        return kt_t[c][:, h * 512 : (h + 1) * 512]

    # ---- wavefront: s0,s1,s2 full + s3 first half accumulate per arriving
    # d-tile. At slow (pre-HAM-flip) clock the PE falls behind the DMA; the
    # backlog it drains after load-end bridges the softmax handoff. ----
    sp0 = psum_big.tile([P, S], F32, tag="sp", name="sp0")
    sp1 = psum_big.tile([P, S], F32, tag="sp", name="sp1")
    sp2 = psum_big.tile([P, S], F32, tag="sp", name="sp2")
    sps = (sp0, sp1, sp2)
    s3a = psum_half.tile([P, 512], F32, tag="oph", name="s3a")
    for c in range(NT):
        for i in (0, 1, 2):
            lhsT = q_lhsT(c, i)
            for h in range(2):
                nc.tensor.matmul(
                    sps[i][:, h * 512 : (h + 1) * 512],
                    lhsT,
                    k_rhs_half(c, h),
                    start=(c == 0),
                    stop=(c == NT - 1),
                )
        nc.tensor.matmul(
            s3a,
            q_lhsT(c, 3),
            k_rhs_half(c, 0),
            start=(c == 0),
            stop=(c == NT - 1),
        )
    # bridge: s3's second half, real work while softmax(s0) runs
    s3b = psum_half.tile([P, 512], F32, tag="oph", name="s3b")
    for c in range(NT):
        nc.tensor.matmul(
            s3b,
            q_lhsT(c, 3),
            k_rhs_half(c, 1),
            start=(c == 0),
            stop=(c == NT - 1),
        )

    def stage_softmax(i, sp):
        negmax = smalls.tile([P, 1], F32, tag="negmax", name=f"negmax{i}")
        nc.vector.reduce_max(
            out=negmax, in_=sp, axis=mybir.AxisListType.X, negate=True
        )
        pexp = ppool.tile([P, S], F16, tag="pexp", name=f"pexp{i}")
        sumexp = smalls.tile([P, 1], F32, tag="sumexp", name=f"sumexp{i}")
        nc.scalar.activation(
            out=pexp, in_=sp, func=EXP, bias=negmax, scale=1.0, accum_out=sumexp
        )
        return pexp, sumexp

    def stage_softmax_halves(ha, hb):
        m0 = smalls.tile([P, 1], F32, tag="negmax", name="m3a")
        nc.vector.reduce_max(out=m0, in_=ha, axis=mybir.AxisListType.X, negate=True)
        m1 = smalls.tile([P, 1], F32, tag="negmax", name="m3b")
        nc.vector.reduce_max(out=m1, in_=hb, axis=mybir.AxisListType.X, negate=True)
        negmax = smalls.tile([P, 1], F32, tag="negmax", name="m3")
        nc.vector.tensor_tensor(out=negmax, in0=m0, in1=m1, op=AluOpType.min)
        pexp = ppool.tile([P, S], F16, tag="pexp", name="pexp3")
        se0 = smalls.tile([P, 1], F32, tag="sumexp", name="se3a")
        nc.scalar.activation(
            out=pexp[:, 0:512], in_=ha, func=EXP, bias=negmax, scale=1.0,
            accum_out=se0,
        )
        se1 = smalls.tile([P, 1], F32, tag="sumexp", name="se3b")
        nc.scalar.activation(
            out=pexp[:, 512:1024], in_=hb, func=EXP, bias=negmax, scale=1.0,
            accum_out=se1,
        )
        sumexp = smalls.tile([P, 1], F32, tag="sumexp", name="sumexp3")
        nc.vector.tensor_add(out=sumexp, in0=se0, in1=se1)
        return pexp, sumexp

    def stage_ptT(i, pexp):
        """P^T via the DMA xbar: one transpose DMA per s-tile. Out AP
        [p, j, s] scatters each 128x128 block transposed in place."""
        pt = ptpool.tile([P, S], F16, tag="pt", name=f"pt{i}")
        nc.sync.dma_start(
            out=pt[:, :].rearrange("p (j s) -> p j s", j=NT),
            in_=pexp[:, :],
            transpose=True,
        )
        return pt

    def stage_scores(i):
        sp = psum_big.tile([P, S], F32, tag="sp", name=f"sp{i}")
        for c in range(NT):
            lhsT = q_lhsT(c, i)
            for h in range(2):
                nc.tensor.matmul(
                    sp[:, h * 512 : (h + 1) * 512],
                    lhsT,
                    k_rhs_half(c, h),
                    start=(c == 0),
                    stop=(c == NT - 1),
                )
        return sp

    def stage_av(i, pt, sumexp):
        # Each half is its own PSUM tile + SBUF tile: the half-h normalize
        # and store overlap the half-(h+1) matmuls with no false WAR deps.
        recip = smalls.tile([P, 1], F32, tag="recip", name=f"recip{i}")
        nc.vector.reciprocal(out=recip, in_=sumexp)
        for h in range(2):
            op = psum_half.tile([P, 512], F32, tag="oph", name=f"op{i}_{h}")
            ot = outpool.tile([P, 512], F16, tag="ot", name=f"ot{i}_{h}")
            for j in range(NT):
                nc.tensor.matmul(
                    op,
                    pt[:, j * P : (j + 1) * P],
                    vf_t[j][:, h * 512 : (h + 1) * 512],
                    start=(j == 0),
                    stop=(j == NT - 1),
                )
            # normalize on ACT (per-partition scale); DVE stays light
            nc.scalar.activation(out=ot, in_=op, func=COPY, scale=recip)
            nc.sync.dma_start(
                out=out_d[i * P : (i + 1) * P, h * 512 : (h + 1) * 512], in_=ot
            )

    # ---- schedule. softmax/AV order: 0, 3, 1, 2, 4..7 (s3 early frees the
    # half banks for AV0; matching AV order keeps pt lifetimes short).
    sm = {}
    pts = {}
    sm[0] = stage_softmax(0, sp0)
    pts[0] = stage_ptT(0, sm[0][0])
    sm[3] = stage_softmax_halves(s3a, s3b)
    pts[3] = stage_ptT(3, sm[3][0])
    sm[1] = stage_softmax(1, sp1)
    pts[1] = stage_ptT(1, sm[1][0])
    sm[2] = stage_softmax(2, sp2)
    pts[2] = stage_ptT(2, sm[2][0])

    av_order = [0, 3, 1, 2, 4, 5, 6, 7]
    for k, i in enumerate(range(4, NT)):
        sp = stage_scores(i)
        sm[i] = stage_softmax(i, sp)
        pts[i] = stage_ptT(i, sm[i][0])
        j = av_order[k]
        stage_av(j, pts.pop(j), sm.pop(j)[1])
    for j in av_order[NT - 4 :]:
        stage_av(j, pts.pop(j), sm.pop(j)[1])


def _get_program():
    key = "v16b"
    if key not in _CACHE:
        nc = bacc.Bacc("TRN2", num_devices=B)
        from contextlib import ExitStack

        with tile.TileContext(nc) as tc:
            with ExitStack() as ctx:
                _build(ctx, tc)
        nc.compile()
        _CACHE[key] = nc
    return _CACHE[key]


def kernel(query, key, value, braak_embed, braak_stages):
    query = np.asarray(query, dtype=np.float32)
    key_in = np.asarray(key, dtype=np.float32)
    value = np.asarray(value, dtype=np.float32)
    braak_embed = np.asarray(braak_embed, dtype=np.float32)
    stages = np.asarray(braak_stages).astype(np.int64)

    bias16 = braak_embed[stages].astype(np.float16)  # [B, S] host gather
    biasb = np.ascontiguousarray(
        np.broadcast_to(bias16[:, None, :], (B, P, S))
    )  # pre-broadcast across partitions
    # Host marshalling: fp16 casts (the kernel consumes fp16 either way)
    # and layout transposes of Q/K to the d-major layout the PE needs.
    qT16 = np.ascontiguousarray(query.astype(np.float16).transpose(0, 2, 1))
    kT16 = np.ascontiguousarray(key_in.astype(np.float16).transpose(0, 2, 1))
    v16 = np.ascontiguousarray(value.astype(np.float16))

    nc = _get_program()
    in_maps = [
        {
            "qT": qT16[b],
            "kT": kT16[b],
            "v": v16[b],
            "biasb": biasb[b],
        }
        for b in range(B)
    ]
    trace = os.environ.get("BRAAK_TRACE", "0") == "1"
    if trace:
        try:  # tracing needs the NTFF hook; never let it break a run
            from antenv.axon_hooks import get_axon_ntff_profile_hook  # noqa: F401
        except ImportError:
            trace = False
    res = run_bass_kernel_spmd(nc, in_maps, list(range(B)), trace=trace)
    if trace:
        kernel.last_exec_time_ns = res.exec_time_ns
        kernel.last_profile = res
    out = np.stack([res.results[b]["out"] for b in range(B)]).astype(np.float32)
    return out


kernel.last_exec_time_ns = None
kernel.last_profile = None


# revision 13
# speedup vs baseline: 1.1218x; 1.0071x over previous
# Braak-aware attention kernel for Trainium2 (Bass/Tile), 8 NeuronCores.
#
# Problem (per sample b of B=8, all fp32 in HBM):
#   bias[s]   = braak_embed[braak_stages[b], s]          (per-row constant)
#   q'[s,d]   = query[b,s,d] + bias[s]
#   S[s,t]    = sum_d q'[s,d] * key[b,t,d]
#   P         = softmax_t(S)
#   out[s,d]  = sum_t P[s,t] * value[b,t,d]
#
# Sharding: data-parallel, one sample per core (8 samples, 8 cores), no comms.
# The braak_embed gather by integer stage is host-side (pure indexing).
#
# Device strategy (v16; v15 measured 84.4us, see git-less history in comments):
#   Trace model (validated on v15): exec ~= [HAM flip at ~16.6us] +
#   post-flip PE columns / 2.34 cols-per-ns + tail + ~9us fixed postamble
#   (compiler sem-reset sweep). The PE array runs at ~0.9 col/ns before the
#   HAM clock flip (~9.3us after first matmul activity) and 2.34 after, and
#   is the bottleneck end-to-end, so v16 minimizes TOTAL PE columns and
#   keeps the array dense with real work:
#   - P^T via the DMA xbar transpose (one dma_start(transpose=True) per
#     s-tile, out AP [p, j, s]) instead of 64 PE transpose matmuls: -8192
#     PE columns and frees 2 PSUM banks.
#   - Wavefront accumulates 3.5 s-tiles (s0-s2 full + s3 half) in all 8
#     PSUM banks as Q^T/K^T d-tiles stream in. The leftover matmul backlog
#     at load-end plus the s3-second-half bridge REPLACES the old filler
#     matmuls: the array never idles at the wavefront->softmax handoff, so
#     the HAM gate stays up without burning ~12k filler columns.
#   - bias ships as [1, S] fp16 (2KB) and is partition-broadcast on GPSIMD,
#     taking 254KB off the DMA-capped load phase (both hw queues run at the
#     ~358 GB/s/core HBM cap during the load; Q/K land ~0.7us earlier).
#   - softmax order s0, s3, s1, s2 frees the two half banks for the first
#     AV quickly; AVs run in the same order. AV accumulates fp32 in
#     [128,512] half-bank tiles, double-buffered so the ACT normalize
#     (COPY x 1/rowsum) of half h overlaps the next half's matmuls.
#   - steady state alternates scores(s4..s7) and AVs on the PE; softmax
#     (DVE max / ACT exp) and the P^T DMA ride under them.
# Numerics: fp16 rounding of Q'/K dominates (~2.4e-3 output rel-L2 vs the
# fp32 reference; threshold 2e-2). The DMA transpose is exact byte movement.

import os
import sys

for _p in ("/opt/trn_rl_repo",):
    if _p not in sys.path:
        sys.path.insert(0, _p)

import numpy as np

import concourse.bass as bass
import concourse.tile as tile
from concourse import bacc, mybir
from concourse.bass_utils import run_bass_kernel_spmd

B, S, D = 8, 1024, 1024
P = 128
NT = S // P  # 8 tiles of 128 along every axis
F32 = mybir.dt.float32
F16 = mybir.dt.float16
EXP = mybir.ActivationFunctionType.Exp
COPY = mybir.ActivationFunctionType.Copy

N_WARM = 16  # identity warmup matmuls: start PE activity (HAM ramp) early


_CACHE = {}


def _build(ctx, tc):
    from concourse.alu_op_type import AluOpType

    nc = tc.nc
    qT_d = nc.dram_tensor("qT", [D, S], F16, kind="ExternalInput").ap()
    kT_d = nc.dram_tensor("kT", [D, S], F16, kind="ExternalInput").ap()
    v_d = nc.dram_tensor("v", [S, D], F16, kind="ExternalInput").ap()
    # bias pre-broadcast to 128 partitions host-side (a GPSIMD
    # partition_broadcast was tried: its ucode lib-load + drain serialized
    # ~13us before the first qb add — far worse than the 254KB of DMA)
    bias_d = nc.dram_tensor("biasb", [P, S], F16, kind="ExternalInput").ap()
    out_d = nc.dram_tensor("out", [S, D], F16, kind="ExternalOutput").ap()

    const = ctx.enter_context(tc.tile_pool(name="const", bufs=1))
    wts = ctx.enter_context(tc.tile_pool(name="wts", bufs=1))
    ppool = ctx.enter_context(tc.tile_pool(name="ppool", bufs=3))
    ptpool = ctx.enter_context(tc.tile_pool(name="ptpool", bufs=4))
    outpool = ctx.enter_context(tc.tile_pool(name="outpool", bufs=4))
    smalls = ctx.enter_context(tc.tile_pool(name="smalls", bufs=3))
    # all 8 PSUM banks: 3 x [128,1024] full score tiles + 2 x [128,512]
    # half-bank tiles (s3's two halves during the wavefront, AV halves after)
    psum_big = ctx.enter_context(tc.tile_pool(name="psum_big", bufs=3, space="PSUM"))
    psum_half = ctx.enter_context(tc.tile_pool(name="psum_half", bufs=2, space="PSUM"))

    # ---- constants; memset-fed warmup source lets PE warmup start in the
    # preamble without waiting on any DMA ----
    wsrc = const.tile([P, P], F16, tag="wsrc")
    nc.vector.memset(wsrc, 0.25)
    bias_bc = const.tile([P, S], F16, tag="bias_bc")
    nc.sync.dma_start(out=bias_bc, in_=bias_d)

    # ---- PE warmup (no DMA deps): starts the HAM clock ramp ASAP. Writes
    # into the first psum_big slot; the wavefront's s2 reclaims it later.
    warm = psum_big.tile([P, S], F32, tag="sp", name="warm")
    for w in range(N_WARM):
        nc.tensor.matmul(
            warm[:, 0:P], wsrc, wsrc, start=(w == 0), stop=(w == N_WARM - 1)
        )

    # ---- persistent operands, one tile per 128-row d/t-tile: Tile deps are
    # tile-granular, and per-tile DMAs keep many transfers in flight (the
    # two hw queues together run at the per-core HBM cap ~358 GB/s). ----
    kt_t = [wts.tile([P, S], F16, tag=f"kt{c}", name=f"kt{c}") for c in range(NT)]
    qraw_t = [
        wts.tile([P, S], F16, tag=f"qraw{c}", name=f"qraw{c}") for c in range(NT)
    ]
    qb_t = [wts.tile([P, S], F16, tag=f"qb{c}", name=f"qb{c}") for c in range(NT)]
    vf_t = [wts.tile([P, D], F16, tag=f"vf{j}", name=f"vf{j}") for j in range(NT)]

    for c in range(NT):
        nc.scalar.dma_start(out=kt_t[c], in_=kT_d[c * P : (c + 1) * P, :])
        nc.sync.dma_start(out=qraw_t[c], in_=qT_d[c * P : (c + 1) * P, :])
        nc.vector.tensor_add(out=qb_t[c], in0=qraw_t[c], in1=bias_bc)
    # (no actwarm: with the first ACTIVATE now being exp0, the scheduler
    # places ACT_TABLE_LOAD after the kt/v DMA issues — off the kt stream's
    # critical path, and still well before the first Exp. An early actwarm
    # COPY pulled the 1.3us table load AHEAD of the kt issues instead.)
    # V split across both hw queues BEHIND qk (FIFO keeps qk first)
    for j in range(NT):
        eng = nc.sync if j % 2 == 0 else nc.scalar
        eng.dma_start(out=vf_t[j], in_=v_d[j * P : (j + 1) * P, :])

    def q_lhsT(c, i):
        return qb_t[c][:, i * P : (i + 1) * P]

    def k_rhs_half(c, h):
        return kt_t[c][:, h * 512 : (h + 1) * 512]

    # ---- wavefront: s0,s1 (+ s2 shrinking) accumulate per arriving d-tile.
    # At slow (pre-HAM-flip) clock the PE falls behind the DMA, so ~10k
    # columns of s2-tail/s3 work are DEFERRED to after the last d-tile:
    # s0's final matmuls then run the moment c7 lands (softmax s0 starts
    # ~4us earlier than with a c-major emission), and the deferred drain is
    # the real-work bridge that keeps the HAM clock up during softmax. ----
    sp0 = psum_big.tile([P, S], F32, tag="sp", name="sp0")
    sp1 = psum_big.tile([P, S], F32, tag="sp", name="sp1")
    sp2 = psum_big.tile([P, S], F32, tag="sp", name="sp2")
    sps = (sp0, sp1, sp2)
    for c in range(NT):
        for i in (0, 1):
            lhsT = q_lhsT(c, i)
            for h in range(2):
                nc.tensor.matmul(
                    sps[i][:, h * 512 : (h + 1) * 512],
                    lhsT,
                    k_rhs_half(c, h),
                    start=(c == 0),
                    stop=(c == NT - 1),
                )
        nc.tensor.matmul(
            sp2[:, 0:512],
            q_lhsT(c, 2),
            k_rhs_half(c, 0),
            start=(c == 0),
            stop=(c == NT - 1),
        )
        if c < 4:
            nc.tensor.matmul(
                sp2[:, 512:1024],
                q_lhsT(c, 2),
                k_rhs_half(c, 1),
                start=(c == 0),
                stop=False,
            )
    # deferred tail: s2's second-half remainder, then s3's two halves
    for c in range(4, NT):
        nc.tensor.matmul(
            sp2[:, 512:1024],
            q_lhsT(c, 2),
            k_rhs_half(c, 1),
            start=False,
            stop=(c == NT - 1),
        )
    s3a = psum_half.tile([P, 512], F32, tag="oph", name="s3a")
    for c in range(NT):
        nc.tensor.matmul(
            s3a,
            q_lhsT(c, 3),
            k_rhs_half(c, 0),
            start=(c == 0),
            stop=(c == NT - 1),
        )
    s3b = psum_half.tile([P, 512], F32, tag="oph", name="s3b")
    for c in range(NT):
        nc.tensor.matmul(
            s3b,
            q_lhsT(c, 3),
            k_rhs_half(c, 1),
            start=(c == 0),
            stop=(c == NT - 1),
        )

    def stage_softmax(i, sp):
        negmax = smalls.tile([P, 1], F32, tag="negmax", name=f"negmax{i}")
        nc.vector.reduce_max(
            out=negmax, in_=sp, axis=mybir.AxisListType.X, negate=True
        )
        pexp = ppool.tile([P, S], F16, tag="pexp", name=f"pexp{i}")
        sumexp = smalls.tile([P, 1], F32, tag="sumexp", name=f"sumexp{i}")
        nc.scalar.activation(
            out=pexp, in_=sp, func=EXP, bias=negmax, scale=1.0, accum_out=sumexp
        )
        # reciprocal here (not in stage_av): keeps it ahead of later
        # reduce_maxes in the strict-FIFO DVE queue
        recip = smalls.tile([P, 1], F32, tag="recip", name=f"recip{i}")
        nc.vector.reciprocal(out=recip, in_=sumexp)
        return pexp, recip

    def stage_softmax_halves(ha, hb):
        m0 = smalls.tile([P, 1], F32, tag="negmax", name="m3a")
        nc.vector.reduce_max(out=m0, in_=ha, axis=mybir.AxisListType.X, negate=True)
        m1 = smalls.tile([P, 1], F32, tag="negmax", name="m3b")
        nc.vector.reduce_max(out=m1, in_=hb, axis=mybir.AxisListType.X, negate=True)
        negmax = smalls.tile([P, 1], F32, tag="negmax", name="m3")
        nc.vector.tensor_tensor(out=negmax, in0=m0, in1=m1, op=AluOpType.min)
        pexp = ppool.tile([P, S], F16, tag="pexp", name="pexp3")
        se0 = smalls.tile([P, 1], F32, tag="sumexp", name="se3a")
        nc.scalar.activation(
            out=pexp[:, 0:512], in_=ha, func=EXP, bias=negmax, scale=1.0,
            accum_out=se0,
        )
        se1 = smalls.tile([P, 1], F32, tag="sumexp", name="se3b")
        nc.scalar.activation(
            out=pexp[:, 512:1024], in_=hb, func=EXP, bias=negmax, scale=1.0,
            accum_out=se1,
        )
        sumexp = smalls.tile([P, 1], F32, tag="sumexp", name="sumexp3")
        nc.vector.tensor_add(out=sumexp, in0=se0, in1=se1)
        recip = smalls.tile([P, 1], F32, tag="recip", name="recip3")
        nc.vector.reciprocal(out=recip, in_=sumexp)
        return pexp, recip

    def stage_ptT(i, pexp):
        """P^T via the DMA xbar: one transpose DMA per s-tile. Out AP
        [p, j, s] scatters each 128x128 block transposed in place."""
        pt = ptpool.tile([P, S], F16, tag="pt", name=f"pt{i}")
        nc.sync.dma_start(
            out=pt[:, :].rearrange("p (j s) -> p j s", j=NT),
            in_=pexp[:, :],
            transpose=True,
        )
        return pt

    def stage_scores(i):
        sp = psum_big.tile([P, S], F32, tag="sp", name=f"sp{i}")
        for c in range(NT):
            lhsT = q_lhsT(c, i)
            for h in range(2):
                nc.tensor.matmul(
                    sp[:, h * 512 : (h + 1) * 512],
                    lhsT,
                    k_rhs_half(c, h),
                    start=(c == 0),
                    stop=(c == NT - 1),
                )
        return sp

    def stage_av(i, pt, recip):
        # Each half is its own PSUM tile + SBUF tile: the half-h normalize
        # and store overlap the half-(h+1) matmuls with no false WAR deps.
        for h in range(2):
            op = psum_half.tile([P, 512], F32, tag="oph", name=f"op{i}_{h}")
            ot = outpool.tile([P, 512], F16, tag="ot", name=f"ot{i}_{h}")
            for j in range(NT):
                nc.tensor.matmul(
                    op,
                    pt[:, j * P : (j + 1) * P],
                    vf_t[j][:, h * 512 : (h + 1) * 512],
                    start=(j == 0),
                    stop=(j == NT - 1),
                )
            # normalize on ACT (per-partition scale); DVE stays light
            nc.scalar.activation(out=ot, in_=op, func=COPY, scale=recip)
            nc.sync.dma_start(
                out=out_d[i * P : (i + 1) * P, h * 512 : (h + 1) * 512], in_=ot
            )

    # ---- schedule: softmaxes in tile order as their scores complete
    # (s0/s1 at load-end, s2 and s3 after the deferred drain), then
    # scores(s4..s7) and AVs alternate on the PE.
    sm = {}
    pts = {}
    sm[0] = stage_softmax(0, sp0)
    pts[0] = stage_ptT(0, sm[0][0])
    sm[1] = stage_softmax(1, sp1)
    pts[1] = stage_ptT(1, sm[1][0])
    sm[2] = stage_softmax(2, sp2)
    pts[2] = stage_ptT(2, sm[2][0])
    sm[3] = stage_softmax_halves(s3a, s3b)
    pts[3] = stage_ptT(3, sm[3][0])

    for i in range(4, NT):
        sp = stage_scores(i)
        sm[i] = stage_softmax(i, sp)
        pts[i] = stage_ptT(i, sm[i][0])
        j = i - 4
        stage_av(j, pts.pop(j), sm.pop(j)[1])
    for j in range(NT - 4, NT):
        stage_av(j, pts.pop(j), sm.pop(j)[1])


def _get_program():
    key = "v17"
    if key not in _CACHE:
        nc = bacc.Bacc("TRN2", num_devices=B)
        from contextlib import ExitStack

        with tile.TileContext(nc) as tc:
            with ExitStack() as ctx:
                _build(ctx, tc)
        nc.compile()
        _CACHE[key] = nc
    return _CACHE[key]


def kernel(query, key, value, braak_embed, braak_stages):
    query = np.asarray(query, dtype=np.float32)
    key_in = np.asarray(key, dtype=np.float32)
    value = np.asarray(value, dtype=np.float32)
    braak_embed = np.asarray(braak_embed, dtype=np.float32)
    stages = np.asarray(braak_stages).astype(np.int64)

    bias16 = braak_embed[stages].astype(np.float16)  # [B, S] host gather
    biasb = np.ascontiguousarray(
        np.broadcast_to(bias16[:, None, :], (B, P, S))
    )  # pre-broadcast across partitions
    # Host marshalling: fp16 casts (the kernel consumes fp16 either way)
    # and layout transposes of Q/K to the d-major layout the PE needs.
    qT16 = np.ascontiguousarray(query.astype(np.float16).transpose(0, 2, 1))
    kT16 = np.ascontiguousarray(key_in.astype(np.float16).transpose(0, 2, 1))
    v16 = np.ascontiguousarray(value.astype(np.float16))

    nc = _get_program()
    in_maps = [
        {
            "qT": qT16[b],
            "kT": kT16[b],
            "v": v16[b],
            "biasb": biasb[b],
        }
        for b in range(B)
    ]
    trace = os.environ.get("BRAAK_TRACE", "0") == "1"
    if trace:
        try:  # tracing needs the NTFF hook; never let it break a run
            from antenv.axon_hooks import get_axon_ntff_profile_hook  # noqa: F401
        except ImportError:
            trace = False
    res = run_bass_kernel_spmd(nc, in_maps, list(range(B)), trace=trace)
    if trace:
        kernel.last_exec_time_ns = res.exec_time_ns
        kernel.last_profile = res
    out = np.stack([res.results[b]["out"] for b in range(B)]).astype(np.float32)
    return out


kernel.last_exec_time_ns = None
kernel.last_profile = None


# revision 15
# speedup vs baseline: 1.1414x; 1.0174x over previous
# Braak-aware attention kernel for Trainium2 (Bass/Tile), 8 NeuronCores.
#
# Problem (per sample b of B=8, all fp32 in HBM):
#   bias[s]   = braak_embed[braak_stages[b], s]          (per-row constant)
#   q'[s,d]   = query[b,s,d] + bias[s]
#   S[s,t]    = sum_d q'[s,d] * key[b,t,d]
#   P         = softmax_t(S)
#   out[s,d]  = sum_t P[s,t] * value[b,t,d]
#
# Sharding: data-parallel, one sample per core (8 samples, 8 cores), no comms.
# The braak_embed gather by integer stage is host-side (pure indexing).
#
# Device strategy (v16; v15 measured 84.4us, see git-less history in comments):
#   Trace model (validated on v15): exec ~= [HAM flip at ~16.6us] +
#   post-flip PE columns / 2.34 cols-per-ns + tail + ~9us fixed postamble
#   (compiler sem-reset sweep). The PE array runs at ~0.9 col/ns before the
#   HAM clock flip (~9.3us after first matmul activity) and 2.34 after, and
#   is the bottleneck end-to-end, so v16 minimizes TOTAL PE columns and
#   keeps the array dense with real work:
#   - P^T via the DMA xbar transpose (one dma_start(transpose=True) per
#     s-tile, out AP [p, j, s]) instead of 64 PE transpose matmuls: -8192
#     PE columns and frees 2 PSUM banks.
#   - Wavefront accumulates 3.5 s-tiles (s0-s2 full + s3 half) in all 8
#     PSUM banks as Q^T/K^T d-tiles stream in. The leftover matmul backlog
#     at load-end plus the s3-second-half bridge REPLACES the old filler
#     matmuls: the array never idles at the wavefront->softmax handoff, so
#     the HAM gate stays up without burning ~12k filler columns.
#   - bias ships as [1, S] fp16 (2KB) and is partition-broadcast on GPSIMD,
#     taking 254KB off the DMA-capped load phase (both hw queues run at the
#     ~358 GB/s/core HBM cap during the load; Q/K land ~0.7us earlier).
#   - softmax order s0, s3, s1, s2 frees the two half banks for the first
#     AV quickly; AVs run in the same order. AV accumulates fp32 in
#     [128,512] half-bank tiles, double-buffered so the ACT normalize
#     (COPY x 1/rowsum) of half h overlaps the next half's matmuls.
#   - steady state alternates scores(s4..s7) and AVs on the PE; softmax
#     (DVE max / ACT exp) and the P^T DMA ride under them.
# Numerics: fp16 rounding of Q'/K dominates (~2.4e-3 output rel-L2 vs the
# fp32 reference; threshold 2e-2). The DMA transpose is exact byte movement.

import os
import sys

for _p in ("/opt/trn_rl_repo",):
    if _p not in sys.path:
        sys.path.insert(0, _p)

import numpy as np

import concourse.bass as bass
import concourse.tile as tile
from concourse import bacc, mybir
from concourse.bass_utils import run_bass_kernel_spmd

B, S, D = 8, 1024, 1024
P = 128
NT = S // P  # 8 tiles of 128 along every axis
F32 = mybir.dt.float32
F16 = mybir.dt.float16
EXP = mybir.ActivationFunctionType.Exp
COPY = mybir.ActivationFunctionType.Copy

N_WARM = 16  # identity warmup matmuls: start PE activity (HAM ramp) early


_CACHE = {}


def _build(ctx, tc):
    from concourse.alu_op_type import AluOpType

    nc = tc.nc
    qT_d = nc.dram_tensor("qT", [D, S], F16, kind="ExternalInput").ap()
    kT_d = nc.dram_tensor("kT", [D, S], F16, kind="ExternalInput").ap()
    v_d = nc.dram_tensor("v", [S, D], F16, kind="ExternalInput").ap()
    # bias pre-broadcast to 128 partitions host-side (a GPSIMD
    # partition_broadcast was tried: its ucode lib-load + drain serialized
    # ~13us before the first qb add — far worse than the 254KB of DMA)
    bias_d = nc.dram_tensor("biasb", [P, S], F16, kind="ExternalInput").ap()
    out_d = nc.dram_tensor("out", [S, D], F16, kind="ExternalOutput").ap()

    const = ctx.enter_context(tc.tile_pool(name="const", bufs=1))
    wts = ctx.enter_context(tc.tile_pool(name="wts", bufs=1))
    # generous bufs: a pexp slot-reuse wait once chained ptT0's transfer ->
    # exp3 -> AV0's half-bank, costing ~1.1us of PE idle
    ppool = ctx.enter_context(tc.tile_pool(name="ppool", bufs=4))
    ptpool = ctx.enter_context(tc.tile_pool(name="ptpool", bufs=5))
    outpool = ctx.enter_context(tc.tile_pool(name="outpool", bufs=4))
    smalls = ctx.enter_context(tc.tile_pool(name="smalls", bufs=5))
    # all 8 PSUM banks: 3 x [128,1024] full score tiles + 2 x [128,512]
    # half-bank tiles (s3's two halves during the wavefront, AV halves after)
    psum_big = ctx.enter_context(tc.tile_pool(name="psum_big", bufs=3, space="PSUM"))
    psum_half = ctx.enter_context(tc.tile_pool(name="psum_half", bufs=2, space="PSUM"))

    # ---- constants; memset-fed warmup source lets PE warmup start in the
    # preamble without waiting on any DMA ----
    wsrc = const.tile([P, P], F16, tag="wsrc")
    nc.vector.memset(wsrc, 0.25)
    bias_bc = const.tile([P, S], F16, tag="bias_bc")
    nc.sync.dma_start(out=bias_bc, in_=bias_d)

    # ---- PE warmup (no DMA deps): starts the HAM clock ramp ASAP. Writes
    # into the first psum_big slot; the wavefront's s2 reclaims it later.
    warm = psum_big.tile([P, S], F32, tag="sp", name="warm")
    for w in range(N_WARM):
        nc.tensor.matmul(
            warm[:, 0:P], wsrc, wsrc, start=(w == 0), stop=(w == N_WARM - 1)
        )

    # ---- persistent operands, one tile per 128-row d/t-tile: Tile deps are
    # tile-granular, and per-tile DMAs keep many transfers in flight (the
    # two hw queues together run at the per-core HBM cap ~358 GB/s). ----
    kt_t = [wts.tile([P, S], F16, tag=f"kt{c}", name=f"kt{c}") for c in range(NT)]
    qraw_t = [
        wts.tile([P, S], F16, tag=f"qraw{c}", name=f"qraw{c}") for c in range(NT)
    ]
    qb_t = [wts.tile([P, S], F16, tag=f"qb{c}", name=f"qb{c}") for c in range(NT)]
    vf_t = [wts.tile([P, D], F16, tag=f"vf{j}", name=f"vf{j}") for j in range(NT)]

    for c in range(NT):
        nc.scalar.dma_start(out=kt_t[c], in_=kT_d[c * P : (c + 1) * P, :])
        nc.sync.dma_start(out=qraw_t[c], in_=qT_d[c * P : (c + 1) * P, :])
        nc.vector.tensor_add(out=qb_t[c], in0=qraw_t[c], in1=bias_bc)
    # (no actwarm: with the first ACTIVATE now being exp0, the scheduler
    # places ACT_TABLE_LOAD after the kt/v DMA issues — off the kt stream's
    # critical path, and still well before the first Exp. An early actwarm
    # COPY pulled the 1.3us table load AHEAD of the kt issues instead.)
    # V split across both hw queues BEHIND qk (FIFO keeps qk first)
    for j in range(NT):
        eng = nc.sync if j % 2 == 0 else nc.scalar
        eng.dma_start(out=vf_t[j], in_=v_d[j * P : (j + 1) * P, :])

    def q_lhsT(c, i):
        return qb_t[c][:, i * P : (i + 1) * P]

    def k_rhs_half(c, h):
        return kt_t[c][:, h * 512 : (h + 1) * 512]

    # ---- wavefront: s0,s1 (+ s2 shrinking) accumulate per arriving d-tile.
    # At slow (pre-HAM-flip) clock the PE falls behind the DMA, so ~10k
    # columns of s2-tail/s3 work are DEFERRED to after the last d-tile:
    # s0's final matmuls then run the moment c7 lands (softmax s0 starts
    # ~4us earlier than with a c-major emission), and the deferred drain is
    # the real-work bridge that keeps the HAM clock up during softmax. ----
    sp0 = psum_big.tile([P, S], F32, tag="sp", name="sp0")
    sp1 = psum_big.tile([P, S], F32, tag="sp", name="sp1")
    sp2 = psum_big.tile([P, S], F32, tag="sp", name="sp2")
    sps = (sp0, sp1, sp2)
    for c in range(NT):
        for i in (0, 1):
            lhsT = q_lhsT(c, i)
            for h in range(2):
                nc.tensor.matmul(
                    sps[i][:, h * 512 : (h + 1) * 512],
                    lhsT,
                    k_rhs_half(c, h),
                    start=(c == 0),
                    stop=(c == NT - 1),
                )
        nc.tensor.matmul(
            sp2[:, 0:512],
            q_lhsT(c, 2),
            k_rhs_half(c, 0),
            start=(c == 0),
            stop=(c == NT - 1),
        )
        if c < 4:
            nc.tensor.matmul(
                sp2[:, 512:1024],
                q_lhsT(c, 2),
                k_rhs_half(c, 1),
                start=(c == 0),
                stop=False,
            )
    # deferred tail: s2's second-half remainder, then s3's two halves
    for c in range(4, NT):
        nc.tensor.matmul(
            sp2[:, 512:1024],
            q_lhsT(c, 2),
            k_rhs_half(c, 1),
            start=False,
            stop=(c == NT - 1),
        )
    s3a = psum_half.tile([P, 512], F32, tag="oph", name="s3a")
    for c in range(NT):
        nc.tensor.matmul(
            s3a,
            q_lhsT(c, 3),
            k_rhs_half(c, 0),
            start=(c == 0),
            stop=(c == NT - 1),
        )
    s3b = psum_half.tile([P, 512], F32, tag="oph", name="s3b")
    for c in range(NT):
        nc.tensor.matmul(
            s3b,
            q_lhsT(c, 3),
            k_rhs_half(c, 1),
            start=(c == 0),
            stop=(c == NT - 1),
        )

    def stage_softmax(i, sp):
        negmax = smalls.tile([P, 1], F32, tag="negmax", name=f"negmax{i}")
        nc.vector.reduce_max(
            out=negmax, in_=sp, axis=mybir.AxisListType.X, negate=True
        )
        pexp = ppool.tile([P, S], F16, tag="pexp", name=f"pexp{i}")
        sumexp = smalls.tile([P, 1], F32, tag="sumexp", name=f"sumexp{i}")
        nc.scalar.activation(
            out=pexp, in_=sp, func=EXP, bias=negmax, scale=1.0, accum_out=sumexp
        )
        # reciprocal here (not in stage_av): keeps it ahead of later
        # reduce_maxes in the strict-FIFO DVE queue
        recip = smalls.tile([P, 1], F32, tag="recip", name=f"recip{i}")
        nc.vector.reciprocal(out=recip, in_=sumexp)
        return pexp, recip

    def stage_softmax_halves(ha, hb):
        m0 = smalls.tile([P, 1], F32, tag="negmax", name="m3a")
        nc.vector.reduce_max(out=m0, in_=ha, axis=mybir.AxisListType.X, negate=True)
        m1 = smalls.tile([P, 1], F32, tag="negmax", name="m3b")
        nc.vector.reduce_max(out=m1, in_=hb, axis=mybir.AxisListType.X, negate=True)
        negmax = smalls.tile([P, 1], F32, tag="negmax", name="m3")
        nc.vector.tensor_tensor(out=negmax, in0=m0, in1=m1, op=AluOpType.min)
        pexp = ppool.tile([P, S], F16, tag="pexp", name="pexp3")
        se0 = smalls.tile([P, 1], F32, tag="sumexp", name="se3a")
        nc.scalar.activation(
            out=pexp[:, 0:512], in_=ha, func=EXP, bias=negmax, scale=1.0,
            accum_out=se0,
        )
        se1 = smalls.tile([P, 1], F32, tag="sumexp", name="se3b")
        nc.scalar.activation(
            out=pexp[:, 512:1024], in_=hb, func=EXP, bias=negmax, scale=1.0,
            accum_out=se1,
        )
        sumexp = smalls.tile([P, 1], F32, tag="sumexp", name="sumexp3")
        nc.vector.tensor_add(out=sumexp, in0=se0, in1=se1)
        recip = smalls.tile([P, 1], F32, tag="recip", name="recip3")
        nc.vector.reciprocal(out=recip, in_=sumexp)
        return pexp, recip

    def stage_ptT(i, pexp):
        """P^T via the DMA xbar: one transpose DMA per s-tile. Out AP
        [p, j, s] scatters each 128x128 block transposed in place."""
        pt = ptpool.tile([P, S], F16, tag="pt", name=f"pt{i}")
        nc.sync.dma_start(
            out=pt[:, :].rearrange("p (j s) -> p j s", j=NT),
            in_=pexp[:, :],
            transpose=True,
        )
        return pt

    def stage_scores(i):
        sp = psum_big.tile([P, S], F32, tag="sp", name=f"sp{i}")
        for c in range(NT):
            lhsT = q_lhsT(c, i)
            for h in range(2):
                nc.tensor.matmul(
                    sp[:, h * 512 : (h + 1) * 512],
                    lhsT,
                    k_rhs_half(c, h),
                    start=(c == 0),
                    stop=(c == NT - 1),
                )
        return sp

    def stage_av(i, pt, recip):
        # Each half is its own PSUM tile + SBUF tile: the half-h normalize
        # and store overlap the half-(h+1) matmuls with no false WAR deps.
        for h in range(2):
            op = psum_half.tile([P, 512], F32, tag="oph", name=f"op{i}_{h}")
            ot = outpool.tile([P, 512], F16, tag="ot", name=f"ot{i}_{h}")
            for j in range(NT):
                nc.tensor.matmul(
                    op,
                    pt[:, j * P : (j + 1) * P],
                    vf_t[j][:, h * 512 : (h + 1) * 512],
                    start=(j == 0),
                    stop=(j == NT - 1),
                )
            # normalize on ACT (per-partition scale); DVE stays light
            nc.scalar.activation(out=ot, in_=op, func=COPY, scale=recip)
            nc.sync.dma_start(
                out=out_d[i * P : (i + 1) * P, h * 512 : (h + 1) * 512], in_=ot
            )

    # ---- schedule: softmaxes in tile order as their scores complete
    # (s0/s1 at load-end, s2 and s3 after the deferred drain), then
    # scores(s4..s7) and AVs alternate on the PE.
    sm = {}
    pts = {}
    sm[0] = stage_softmax(0, sp0)
    pts[0] = stage_ptT(0, sm[0][0])
    sm[1] = stage_softmax(1, sp1)
    pts[1] = stage_ptT(1, sm[1][0])
    sm[2] = stage_softmax(2, sp2)
    pts[2] = stage_ptT(2, sm[2][0])
    sm[3] = stage_softmax_halves(s3a, s3b)
    pts[3] = stage_ptT(3, sm[3][0])

    for i in range(4, NT):
        sp = stage_scores(i)
        sm[i] = stage_softmax(i, sp)
        pts[i] = stage_ptT(i, sm[i][0])
        j = i - 4
        stage_av(j, pts.pop(j), sm.pop(j)[1])
    for j in range(NT - 4, NT):
        stage_av(j, pts.pop(j), sm.pop(j)[1])


def _get_program():
    key = "v17b"
    if key not in _CACHE:
        nc = bacc.Bacc("TRN2", num_devices=B)
        from contextlib import ExitStack

        with tile.TileContext(nc) as tc:
            with ExitStack() as ctx:
                _build(ctx, tc)
        nc.compile()
        _CACHE[key] = nc
    return _CACHE[key]


def kernel(query, key, value, braak_embed, braak_stages):
    query = np.asarray(query, dtype=np.float32)
    key_in = np.asarray(key, dtype=np.float32)
    value = np.asarray(value, dtype=np.float32)
    braak_embed = np.asarray(braak_embed, dtype=np.float32)
    stages = np.asarray(braak_stages).astype(np.int64)

    bias16 = braak_embed[stages].astype(np.float16)  # [B, S] host gather
    biasb = np.ascontiguousarray(
        np.broadcast_to(bias16[:, None, :], (B, P, S))
    )  # pre-broadcast across partitions
    # Host marshalling: fp16 casts (the kernel consumes fp16 either way)
    # and layout transposes of Q/K to the d-major layout the PE needs.
    qT16 = np.ascontiguousarray(query.astype(np.float16).transpose(0, 2, 1))
    kT16 = np.ascontiguousarray(key_in.astype(np.float16).transpose(0, 2, 1))
    v16 = np.ascontiguousarray(value.astype(np.float16))

    nc = _get_program()
    in_maps = [
        {
            "qT": qT16[b],
            "kT": kT16[b],
            "v": v16[b],
            "biasb": biasb[b],
        }
        for b in range(B)
    ]
    trace = os.environ.get("BRAAK_TRACE", "0") == "1"
    if trace:
        try:  # tracing needs the NTFF hook; never let it break a run
            from antenv.axon_hooks import get_axon_ntff_profile_hook  # noqa: F401
        except ImportError:
            trace = False
    res = run_bass_kernel_spmd(nc, in_maps, list(range(B)), trace=trace)
    if trace:
        kernel.last_exec_time_ns = res.exec_time_ns
        kernel.last_profile = res
    out = np.stack([res.results[b]["out"] for b in range(B)]).astype(np.float32)
    return out


kernel.last_exec_time_ns = None
kernel.last_profile = None
